# revision 16
# baseline (speedup 1.0000x reference)
"""Trainium2 Bass kernel for the KBLN scoring model.

Computes, for full inputs:
    score_l = (emb_e[e1] * emb_rel[rel]) @ emb_e.T                       (B, E)
    phi     = exp(-((lit[e1][:,None,:] - lit[None,:,:]) - c)^2 / var)    (B, E, L)
    score_n = einsum('bel,bl->be', phi, nf_weights[rel])
    out     = sigmoid(score_l + score_n)

Reformulation
-------------
With alpha[b,l] = (lit[e1[b],l] - 0.5 - c[l]) / sqrt(var[l]),
     beta[e,l]  = (lit[e,l]    - 0.5)        / sqrt(var[l]),
     g[l]       = -c[l] / sqrt(var[l]):

    phi = exp(-alpha^2) * exp(-(beta-g)^2 + g^2) * exp(x),
    x   = 2*(alpha-g)*beta,  |x| <= 0.5/var[l]  (per-l bound r_l <= 1).

exp(x) is replaced by a per-l degree-3 Chebyshev-interpolant polynomial on
[-r_l, r_l] (near-minimax), which makes score_n a matmul with contraction
4*64 = 256:

    score_n[b,e] = sum_{k,l} A[b,(k,l)] * Bt[(k,l),e]
    A[b,(k,l)]  = w * exp(-alpha^2) * C[k,l] * (2*(alpha-g))^k    (host, tiny)
    Bt[(k,l),e] = exp(-(beta-g)^2 + g^2) * beta^k                 (host, E*L)

score_l folds in as 200 extra contraction rows.  All feature construction
happens on host (it is O(E*L), tiny next to the O(B*E*L) matmul), so the
device program is pure DMA + matmul + sigmoid.

Precision split (tolerance is 2e-2; this lands ~7e-3):
  - k0/k1 polynomial rows (the dominant terms): bf16 x bf16 matmul
  - k2/k3 rows + all 200 emb rows: fp8(e4m3) on BOTH sides with a per-row
    joint rescale a_r = sqrt(max|rhs_r| / max|lhs_r|) (scales cancel in the
    product), packed as TWO DoubleRow matmuls at 0.5 cyc/row:
    128x2 rows (k2k3 | emb 0:128) and 36x2 rows (emb 128:164 | 164:200)
  - output: sigmoid encoded as uint8 via round(127.49*tanh(x/2) + 127.5)
    (decoded on host), quartering the store traffic vs f32

The entity axis is padded 1875 -> 1888 per core so every SBUF byte offset
the PE reads (slice starts, DoubleRow group stride) is 32B-aligned in
every dtype -- the tensor engine silently reads garbage at unaligned
per-partition offsets.

Device schedule: dummy warmup matmuls ramp the PE clock while inputs
stream in (slice-quarter granularity, need-ordered, spread over the sync /
gpsimd / scalar DMA queues); real matmuls run slice-by-slice so each PSUM
bank's tanh (scalar), u8 pack (vector) and store (sync/gpsimd) overlap
the remaining matmuls.

Sharding: entities (E=15000) split evenly across 8 cores (1875 each);
batch side replicated; outputs concatenated on host.
"""

import sys

import numpy as np
import ml_dtypes

for _p in ("/opt/trn_rl_repo", "/root/.axon_site/_ro/trn_rl_repo"):
    if _p not in sys.path:
        sys.path.append(_p)

import concourse.bass as bass
import concourse.bacc as bacc
import concourse.mybir as mybir
from concourse import tile
from concourse import bass_utils

B, E, R, D, L = 256, 15000, 237, 200, 64
NCORES = 8
ES = E // NCORES          # 1875 entities per core
ESP = 1888                # padded per-core entity count (32B aligned all dtypes)
DEG = 3                   # polynomial degree for exp(x) on [-r_l, r_l]
KT = DEG + 1              # 4 polynomial terms
F32 = mybir.dt.float32
BF16 = mybir.dt.bfloat16
F8 = mybir.dt.float8e4
U8 = mybir.dt.uint8
BF16_NP = ml_dtypes.bfloat16
F8_NP = ml_dtypes.float8_e4m3fn
NF8 = 328                 # fp8 contraction rows: k2/k3 (128) + emb (200)
N_SLICES = [(0, 512), (512, 512), (1024, 512), (1504, 384)]
QUARTERS = [(0, 512), (512, 512), (1024, 512), (1536, 352)]
N_WARM = 24               # dummy matmuls to ramp the PE clock
OSC, OBI = 127.49, 127.5  # u8 encode: round(OSC*tanh(x/2) + OBI)

TRACE = False             # test.py sets True to collect an NTFF profile
LAST = None               # last BassKernelResults (for test.py)

_PROG = None              # cached Bass program
_CHEB = None              # cached per-l polynomial coefficients


def _build_program():
    nc = bacc.Bacc("TRN2", target_bir_lowering=False, debug=False)

    rhs0_d = nc.dram_tensor("rhs0", [128, ESP], BF16, kind="ExternalInput")
    rhs12_d = nc.dram_tensor("rhs12", [128, 2, ESP], F8, kind="ExternalInput")
    rhs3_d = nc.dram_tensor("rhs3", [36, 2, ESP], F8, kind="ExternalInput")
    lhs0_d = nc.dram_tensor("lhs0", [128, B], BF16, kind="ExternalInput")
    lhs12_d = nc.dram_tensor("lhs12", [128, 2, B], F8, kind="ExternalInput")
    lhs3_d = nc.dram_tensor("lhs3", [36, 2, B], F8, kind="ExternalInput")
    out_d = nc.dram_tensor("out", [B, ESP], U8, kind="ExternalOutput")

    AF = mybir.ActivationFunctionType
    OP = mybir.AluOpType
    DR = mybir.MatmulPerfMode.DoubleRow

    with tile.TileContext(nc) as tc:
        with (
            tc.tile_pool(name="persist", bufs=1) as pool,
            tc.tile_pool(name="psum", bufs=1, space="PSUM") as ppool,
            tc.tile_pool(name="outs", bufs=4) as opool,
        ):
            # -- PE warmup: ramp the tensor-engine clock while DMAs run
            warm = pool.tile([128, 128], BF16, name="warm")
            wps = ppool.tile([128, 128], F32, name="wps", tag="wps", bufs=1)
            nc.vector.memset(warm, 0.0)
            for i in range(N_WARM):
                nc.tensor.matmul(wps, warm, warm, start=True, stop=True)

            # -- input DMAs, need-ordered, spread over three queues
            lhs0 = pool.tile([128, B], BF16, name="lhs0")
            lhs12 = pool.tile([128, 2, B], F8, name="lhs12")
            lhs3 = pool.tile([128, 2, B], F8, name="lhs3")
            rhs0 = pool.tile([128, ESP], BF16, name="rhs0")
            rhs12 = pool.tile([128, 2, ESP], F8, name="rhs12")
            rhs3 = pool.tile([128, 2, ESP], F8, name="rhs3")
            nc.sync.dma_start(lhs0, lhs0_d[:, :])
            nc.gpsimd.dma_start(lhs12, lhs12_d[:, :, :])
            nc.gpsimd.dma_start(lhs3[:36], lhs3_d[:, :, :])
            nc.sync.dma_start(rhs3[:36], rhs3_d[:, :, :])
            for q0, qsz in QUARTERS:
                cs = np.s_[q0 : q0 + qsz]
                nc.sync.dma_start(rhs0[:, cs], rhs0_d[:, cs])
                nc.gpsimd.dma_start(rhs12[:, :, cs], rhs12_d[:, :, cs])

            # -- matmul / sigmoid-as-u8 / store, slice by slice
            for si, (n0, nsz) in enumerate(N_SLICES):
                for m in range(2):
                    ms = np.s_[m * 128 : (m + 1) * 128]
                    ns = np.s_[n0 : n0 + nsz]
                    ps = ppool.tile([128, 512], F32, name="ps", tag="ps", bufs=4)
                    nc.tensor.matmul(
                        ps[:, :nsz], lhs0[:, ms], rhs0[:, ns],
                        start=True, stop=False,
                    )
                    nc.tensor.matmul(
                        ps[:, :nsz], lhs3[:36, :, ms], rhs3[:36, :, ns],
                        start=False, stop=False, perf_mode=DR,
                    )
                    nc.tensor.matmul(
                        ps[:, :nsz], lhs12[:, :, ms], rhs12[:, :, ns],
                        start=False, stop=True, perf_mode=DR,
                    )
                    tob = opool.tile([128, 512], BF16, name="tob", tag="tob")
                    nc.scalar.activation(tob[:, :nsz], ps[:, :nsz], AF.Tanh, scale=0.5)
                    ob = opool.tile([128, 512], U8, name="ob", tag="ob")
                    nc.vector.tensor_scalar(
                        ob[:, :nsz], tob[:, :nsz], OSC, OBI, OP.mult, OP.add
                    )
                    nc.sync.dma_start(out_d[ms, ns], ob[:, :nsz])

    nc.compile()
    return nc


def _host_prep(emb_e, emb_rel, nf_weights, lit, c, var, e1, rel):
    global _CHEB
    f64 = np.float64
    e1 = np.asarray(e1).astype(np.int64)
    rel = np.asarray(rel).astype(np.int64)
    lit64 = np.asarray(lit, f64)
    c64 = np.asarray(c, f64)
    var64 = np.asarray(var, f64)

    s = np.sqrt(var64)
    g = -c64 / s
    if _CHEB is None:
        r = 0.5 / var64
        C = np.zeros((KT, L))
        for l in range(L):
            ch = np.polynomial.chebyshev.Chebyshev.interpolate(
                np.exp, DEG, domain=[-r[l], r[l]]
            )
            C[:, l] = ch.convert(kind=np.polynomial.Polynomial).coef
        _CHEB = C
    C = _CHEB

    P = lit64[e1]                                   # (B, L)
    w = np.asarray(nf_weights, f64)[rel]            # (B, L)
    amg = (P - 0.5) / s                             # alpha - g
    alpha = amg + g
    u = w * np.exp(-(alpha**2))                     # (B, L)

    # polynomial-term factors
    A = np.empty((KT * L, B), f64)                  # (k-major rows, B)
    Bt = np.empty((KT * L, E), f64)
    beta = (lit64 - 0.5) / s                        # (E, L)
    V = np.exp(-((beta - g) ** 2) + g**2)           # (E, L)
    t2k = np.ones_like(amg)
    bk = V.copy()
    for k in range(KT):
        A[k * L : (k + 1) * L] = (u * C[k] * t2k).T
        Bt[k * L : (k + 1) * L] = bk.T
        t2k *= 2.0 * amg
        bk *= beta

    X = np.asarray(emb_e, f64)[e1] * np.asarray(emb_rel, f64)[rel]  # (B, D)

    # fp8 rows (k2,k3 + emb) with joint per-row rescale (cancels in product)
    Lr = np.concatenate([A[128:256], X.T], axis=0)        # (NF8, B)
    Rr = np.concatenate([Bt[128:256], np.asarray(emb_e, f64).T], axis=0)  # (NF8, E)
    mL = np.abs(Lr).max(axis=1)
    mR = np.abs(Rr).max(axis=1)
    mL[mL == 0] = 1.0
    mR[mR == 0] = 1.0
    a = np.sqrt(mR / mL)
    Lq = (Lr * a[:, None]).astype(F8_NP)                  # (NF8, B)
    Rq = (Rr / a[:, None]).astype(F8_NP)                  # (NF8, E)

    lhs0 = np.ascontiguousarray(A[:128].astype(BF16_NP))  # (128, B) bf16
    lhs12 = np.stack([Lq[0:128], Lq[128:256]], axis=1)    # (128, 2, B)
    lhs3 = np.stack([Lq[256:292], Lq[292:NF8]], axis=1)   # (36, 2, B)

    rhs0 = np.zeros((128, NCORES, ESP), BF16_NP)
    rhs12 = np.zeros((128, 2, NCORES, ESP), F8_NP)
    rhs3 = np.zeros((36, 2, NCORES, ESP), F8_NP)
    rhs0[:, :, :ES] = Bt[:128].astype(BF16_NP).reshape(128, NCORES, ES)
    rhs12[:, 0, :, :ES] = Rq[0:128].reshape(128, NCORES, ES)
    rhs12[:, 1, :, :ES] = Rq[128:256].reshape(128, NCORES, ES)
    rhs3[:, 0, :, :ES] = Rq[256:292].reshape(36, NCORES, ES)
    rhs3[:, 1, :, :ES] = Rq[292:NF8].reshape(36, NCORES, ES)

    in_maps = []
    for ci in range(NCORES):
        in_maps.append(
            {
                "rhs0": np.ascontiguousarray(rhs0[:, ci]),
                "rhs12": np.ascontiguousarray(rhs12[:, :, ci]),
                "rhs3": np.ascontiguousarray(rhs3[:, :, ci]),
                "lhs0": lhs0,
                "lhs12": lhs12,
                "lhs3": lhs3,
            }
        )
    return in_maps


def kernel(emb_e, emb_rel, nf_weights, lit, c, var, e1, rel):
    global _PROG, LAST
    if _PROG is None:
        _PROG = _build_program()
    in_maps = _host_prep(emb_e, emb_rel, nf_weights, lit, c, var, e1, rel)
    res = bass_utils.run_bass_kernel_spmd(
        _PROG, in_maps, core_ids=list(range(NCORES)), trace=TRACE
    )
    LAST = res
    q = np.concatenate(
        [res.results[ci]["out"][:, :ES] for ci in range(NCORES)], axis=1
    ).astype(np.float32)
    # decode u8: stored = round(OSC*tanh(x/2) + OBI) -> sigmoid = (tanh+1)/2
    t = (q - OBI) / OSC
    return np.clip((t + 1.0) * 0.5, 0.0, 1.0).astype(np.float32)


# revision 17
# speedup vs baseline: 1.1043x; 1.1043x over previous
"""Trainium2 Bass kernel for the KBLN scoring model.

Computes, for full inputs:
    score_l = (emb_e[e1] * emb_rel[rel]) @ emb_e.T                       (B, E)
    phi     = exp(-((lit[e1][:,None,:] - lit[None,:,:]) - c)^2 / var)    (B, E, L)
    score_n = einsum('bel,bl->be', phi, nf_weights[rel])
    out     = sigmoid(score_l + score_n)

Reformulation
-------------
With alpha[b,l] = (lit[e1[b],l] - 0.5 - c[l]) / sqrt(var[l]),
     beta[e,l]  = (lit[e,l]    - 0.5)        / sqrt(var[l]),
     g[l]       = -c[l] / sqrt(var[l]):

    phi = exp(-alpha^2) * exp(-(beta-g)^2 + g^2) * exp(x),
    x   = 2*(alpha-g)*beta,  |x| <= 0.5/var[l]  (per-l bound r_l <= 1).

exp(x) is replaced by a per-l degree-3 Chebyshev-interpolant polynomial on
[-r_l, r_l] (near-minimax), which makes score_n a matmul with contraction
4*64 = 256:

    score_n[b,e] = sum_{k,l} A[b,(k,l)] * Bt[(k,l),e]
    A[b,(k,l)]  = w * exp(-alpha^2) * C[k,l] * (2*(alpha-g))^k    (host, tiny)
    Bt[(k,l),e] = exp(-(beta-g)^2 + g^2) * beta^k                 (host, E*L)

score_l folds in as 200 extra contraction rows.  All feature construction
happens on host (it is O(E*L), tiny next to the O(B*E*L) matmul), so the
device program is pure DMA + matmul + sigmoid.

Precision split (tolerance is 2e-2; this lands ~7e-3):
  - k0/k1 polynomial rows (the dominant terms): bf16 x bf16 matmul
  - k2/k3 rows + all 200 emb rows: fp8(e4m3) on BOTH sides with a per-row
    joint rescale a_r = sqrt(max|rhs_r| / max|lhs_r|) (scales cancel in the
    product), packed as one DoubleRow matmul (2x128 rows: k2k3 | emb 0:128,
    double contraction per pass) plus a 72-row tail matmul
  - output: sigmoid encoded as uint8 via round(127.49*tanh(x/2) + 127.5)
    (decoded on host), quartering the store traffic vs f32

The entity axis is padded 1875 -> 1888 per core so every SBUF byte offset
the PE reads (slice starts, DoubleRow group stride) is 32B-aligned in
every dtype -- the tensor engine silently reads garbage at unaligned
per-partition offsets.

Device schedule: dummy warmup matmuls keep the PE busy (ramping its DVFS
clock) while inputs stream in as column halves, need-ordered and balanced
over the sync and gpsimd DMA queues; real matmuls then run slice-by-slice
with each PSUM bank's tanh (scalar) and u8 pack (vector) overlapping the
remaining matmuls, and stores batched per slice-pair on the sync queue.

Sharding: entities (E=15000) split evenly across 8 cores (1875 each);
batch side replicated; outputs concatenated on host.
"""

import sys

import numpy as np
import ml_dtypes

for _p in ("/opt/trn_rl_repo", "/root/.axon_site/_ro/trn_rl_repo"):
    if _p not in sys.path:
        sys.path.append(_p)

import concourse.bass as bass
import concourse.bacc as bacc
import concourse.mybir as mybir
from concourse import tile
from concourse import bass_utils

B, E, R, D, L = 256, 15000, 237, 200, 64
NCORES = 8
ES = E // NCORES          # 1875 entities per core
ESP = 1888                # padded per-core entity count (32B aligned all dtypes)
DEG = 3                   # polynomial degree for exp(x) on [-r_l, r_l]
KT = DEG + 1              # 4 polynomial terms
F32 = mybir.dt.float32
BF16 = mybir.dt.bfloat16
F8 = mybir.dt.float8e4
U8 = mybir.dt.uint8
BF16_NP = ml_dtypes.bfloat16
F8_NP = ml_dtypes.float8_e4m3fn
NF8 = 328                 # fp8 contraction rows: k2/k3 (128) + emb (200)
N_SLICES = [(0, 512), (512, 512), (1024, 512), (1504, 384)]
HALVES = [(0, 1024), (1024, ESP - 1024)]
N_WARM = 36               # dummy matmuls to ramp the PE clock
OSC, OBI = 127.49, 127.5  # u8 encode: round(OSC*tanh(x/2) + OBI)

TRACE = False             # test.py sets True to collect an NTFF profile
LAST = None               # last BassKernelResults (for test.py)

_PROG = None              # cached Bass program
_CHEB = None              # cached per-l polynomial coefficients


def _build_program():
    nc = bacc.Bacc("TRN2", target_bir_lowering=False, debug=False)

    rhs0_d = nc.dram_tensor("rhs0", [128, ESP], BF16, kind="ExternalInput")
    rhs12_d = nc.dram_tensor("rhs12", [128, 2, ESP], F8, kind="ExternalInput")
    rhs3_d = nc.dram_tensor("rhs3", [128, ESP], F8, kind="ExternalInput")
    lhs0_d = nc.dram_tensor("lhs0", [128, B], BF16, kind="ExternalInput")
    lhsf8_d = nc.dram_tensor("lhsf8", [128, 3, B], F8, kind="ExternalInput")
    out_d = nc.dram_tensor("out", [B, ESP], U8, kind="ExternalOutput")

    AF = mybir.ActivationFunctionType
    OP = mybir.AluOpType
    DR = mybir.MatmulPerfMode.DoubleRow

    with tile.TileContext(nc) as tc:
        with (
            tc.tile_pool(name="persist", bufs=1) as pool,
            tc.tile_pool(name="psum", bufs=1, space="PSUM") as ppool,
            tc.tile_pool(name="outs", bufs=4) as opool,
        ):
            # -- PE warmup: ramp the tensor-engine clock while DMAs run
            warm = pool.tile([128, 128], BF16, name="warm")
            wps = ppool.tile([128, 128], F32, name="wps", tag="wps", bufs=1)
            nc.vector.memset(warm, 0.0)
            for i in range(N_WARM):
                nc.tensor.matmul(wps, warm, warm, start=True, stop=True)

            # -- input DMAs, in need order, balanced across two queues
            lhs0 = pool.tile([128, B], BF16, name="lhs0")
            lhsf8 = pool.tile([128, 3, B], F8, name="lhsf8")
            rhs0 = pool.tile([128, ESP], BF16, name="rhs0")
            rhs12 = pool.tile([128, 2, ESP], F8, name="rhs12")
            rhs3 = pool.tile([128, ESP], F8, name="rhs3")
            (h0, hs0), (h1, hs1) = HALVES
            nc.sync.dma_start(lhs0, lhs0_d[:, :])
            nc.gpsimd.dma_start(lhsf8, lhsf8_d[:, :, :])
            nc.sync.dma_start(rhs0[:, h0 : h0 + hs0], rhs0_d[:, h0 : h0 + hs0])
            nc.gpsimd.dma_start(
                rhs12[:, :, h0 : h0 + hs0], rhs12_d[:, :, h0 : h0 + hs0]
            )
            nc.sync.dma_start(rhs3[:, h0 : h0 + hs0], rhs3_d[:, h0 : h0 + hs0])
            nc.gpsimd.dma_start(rhs0[:, h1 : h1 + hs1], rhs0_d[:, h1 : h1 + hs1])
            nc.sync.dma_start(
                rhs12[:, :, h1 : h1 + hs1], rhs12_d[:, :, h1 : h1 + hs1]
            )
            nc.gpsimd.dma_start(rhs3[:, h1 : h1 + hs1], rhs3_d[:, h1 : h1 + hs1])

            # -- matmul / sigmoid-as-u8, slice by slice; store per slice-pair
            obs = {}
            for si, (n0, nsz) in enumerate(N_SLICES):
                for m in range(2):
                    ms = np.s_[m * 128 : (m + 1) * 128]
                    ns = np.s_[n0 : n0 + nsz]
                    ps = ppool.tile([128, 512], F32, name="ps", tag="ps", bufs=4)
                    nc.tensor.matmul(
                        ps[:, :nsz], lhs0[:, ms], rhs0[:, ns],
                        start=True, stop=False,
                    )
                    nc.tensor.matmul(
                        ps[:, :nsz], lhsf8[:, 0:2, ms], rhs12[:, :, ns],
                        start=False, stop=False, perf_mode=DR,
                    )
                    nc.tensor.matmul(
                        ps[:, :nsz], lhsf8[:72, 2, ms], rhs3[:72, ns],
                        start=False, stop=True,
                    )
                    # u8 rows accumulate in a [128, 1024] buffer per (pair, m)
                    key = (si // 2, m)
                    if key not in obs:
                        obs[key] = opool.tile(
                            [128, 1024], U8, name=f"ob{key[0]}{m}", tag=f"ob{key[0]}{m}", bufs=1
                        )
                    ob = obs[key]
                    off = n0 - (si // 2) * 1024
                    tob = opool.tile([128, 512], BF16, name="tob", tag="tob")
                    nc.scalar.activation(tob[:, :nsz], ps[:, :nsz], AF.Tanh, scale=0.5)
                    nc.vector.tensor_scalar(
                        ob[:, off : off + nsz], tob[:, :nsz], OSC, OBI,
                        OP.mult, OP.add,
                    )
                    if si % 2 == 1:
                        p0 = (si // 2) * 1024
                        psz = n0 + nsz - p0
                        nc.sync.dma_start(
                            out_d[ms, p0 : p0 + psz], ob[:, :psz]
                        )

    nc.compile()
    return nc


def _host_prep(emb_e, emb_rel, nf_weights, lit, c, var, e1, rel):
    global _CHEB
    f64 = np.float64
    e1 = np.asarray(e1).astype(np.int64)
    rel = np.asarray(rel).astype(np.int64)
    lit64 = np.asarray(lit, f64)
    c64 = np.asarray(c, f64)
    var64 = np.asarray(var, f64)

    s = np.sqrt(var64)
    g = -c64 / s
    if _CHEB is None:
        r = 0.5 / var64
        C = np.zeros((KT, L))
        for l in range(L):
            ch = np.polynomial.chebyshev.Chebyshev.interpolate(
                np.exp, DEG, domain=[-r[l], r[l]]
            )
            C[:, l] = ch.convert(kind=np.polynomial.Polynomial).coef
        _CHEB = C
    C = _CHEB

    P = lit64[e1]                                   # (B, L)
    w = np.asarray(nf_weights, f64)[rel]            # (B, L)
    amg = (P - 0.5) / s                             # alpha - g
    alpha = amg + g
    u = w * np.exp(-(alpha**2))                     # (B, L)

    # polynomial-term factors
    A = np.empty((KT * L, B), f64)                  # (k-major rows, B)
    Bt = np.empty((KT * L, E), f64)
    beta = (lit64 - 0.5) / s                        # (E, L)
    V = np.exp(-((beta - g) ** 2) + g**2)           # (E, L)
    t2k = np.ones_like(amg)
    bk = V.copy()
    for k in range(KT):
        A[k * L : (k + 1) * L] = (u * C[k] * t2k).T
        Bt[k * L : (k + 1) * L] = bk.T
        t2k *= 2.0 * amg
        bk *= beta

    X = np.asarray(emb_e, f64)[e1] * np.asarray(emb_rel, f64)[rel]  # (B, D)

    # fp8 rows (k2,k3 + emb) with joint per-row rescale (cancels in product)
    Lr = np.concatenate([A[128:256], X.T], axis=0)        # (NF8, B)
    Rr = np.concatenate([Bt[128:256], np.asarray(emb_e, f64).T], axis=0)  # (NF8, E)
    mL = np.abs(Lr).max(axis=1)
    mR = np.abs(Rr).max(axis=1)
    mL[mL == 0] = 1.0
    mR[mR == 0] = 1.0
    a = np.sqrt(mR / mL)
    Lq = (Lr * a[:, None]).astype(F8_NP)                  # (NF8, B)
    Rq = (Rr / a[:, None]).astype(F8_NP)                  # (NF8, E)

    lhs0 = np.ascontiguousarray(A[:128].astype(BF16_NP))  # (128, B) bf16
    lhsf8 = np.zeros((128, 3, B), F8_NP)
    lhsf8[:, 0, :] = Lq[0:128]
    lhsf8[:, 1, :] = Lq[128:256]
    lhsf8[:72, 2, :] = Lq[256:NF8]

    rhs0 = np.zeros((128, NCORES, ESP), BF16_NP)
    rhs12 = np.zeros((128, 2, NCORES, ESP), F8_NP)
    rhs3 = np.zeros((128, NCORES, ESP), F8_NP)
    rhs0[:, :, :ES] = Bt[:128].astype(BF16_NP).reshape(128, NCORES, ES)
    rhs12[:, 0, :, :ES] = Rq[0:128].reshape(128, NCORES, ES)
    rhs12[:, 1, :, :ES] = Rq[128:256].reshape(128, NCORES, ES)
    rhs3[:72, :, :ES] = Rq[256:NF8].reshape(72, NCORES, ES)

    in_maps = []
    for ci in range(NCORES):
        in_maps.append(
            {
                "rhs0": np.ascontiguousarray(rhs0[:, ci]),
                "rhs12": np.ascontiguousarray(rhs12[:, :, ci]),
                "rhs3": np.ascontiguousarray(rhs3[:, ci]),
                "lhs0": lhs0,
                "lhsf8": lhsf8,
            }
        )
    return in_maps


def kernel(emb_e, emb_rel, nf_weights, lit, c, var, e1, rel):
    global _PROG, LAST
    if _PROG is None:
        _PROG = _build_program()
    in_maps = _host_prep(emb_e, emb_rel, nf_weights, lit, c, var, e1, rel)
    res = bass_utils.run_bass_kernel_spmd(
        _PROG, in_maps, core_ids=list(range(NCORES)), trace=TRACE
    )
    LAST = res
    q = np.concatenate(
        [res.results[ci]["out"][:, :ES] for ci in range(NCORES)], axis=1
    ).astype(np.float32)
    # decode u8: stored = round(OSC*tanh(x/2) + OBI) -> sigmoid = (tanh+1)/2
    t = (q - OBI) / OSC
    return np.clip((t + 1.0) * 0.5, 0.0, 1.0).astype(np.float32)


# revision 19
# speedup vs baseline: 1.1231x; 1.0169x over previous
"""Trainium2 Bass kernel for the KBLN scoring model.

Computes, for full inputs:
    score_l = (emb_e[e1] * emb_rel[rel]) @ emb_e.T                       (B, E)
    phi     = exp(-((lit[e1][:,None,:] - lit[None,:,:]) - c)^2 / var)    (B, E, L)
    score_n = einsum('bel,bl->be', phi, nf_weights[rel])
    out     = sigmoid(score_l + score_n)

Reformulation
-------------
With alpha[b,l] = (lit[e1[b],l] - 0.5 - c[l]) / sqrt(var[l]),
     beta[e,l]  = (lit[e,l]    - 0.5)        / sqrt(var[l]),
     g[l]       = -c[l] / sqrt(var[l]):

    phi = exp(-alpha^2) * exp(-(beta-g)^2 + g^2) * exp(x),
    x   = 2*(alpha-g)*beta,  |x| <= 0.5/var[l]  (per-l bound r_l <= 1).

exp(x) is replaced by a per-l degree-3 Chebyshev-interpolant polynomial on
[-r_l, r_l] (near-minimax), which makes score_n a matmul with contraction
4*64 = 256:

    score_n[b,e] = sum_{k,l} A[b,(k,l)] * Bt[(k,l),e]
    A[b,(k,l)]  = w * exp(-alpha^2) * C[k,l] * (2*(alpha-g))^k    (host, tiny)
    Bt[(k,l),e] = exp(-(beta-g)^2 + g^2) * beta^k                 (host, E*L)

score_l folds in as 200 extra contraction rows.  All feature construction
happens on host (it is O(E*L), tiny next to the O(B*E*L) matmul), so the
device program is pure DMA + matmul + sigmoid.

Precision split (tolerance is 2e-2; this lands ~7e-3):
  - k0/k1 polynomial rows (the dominant terms): bf16 x bf16 matmul
  - k2/k3 rows + all 200 emb rows: fp8(e4m3) on BOTH sides with a per-row
    joint rescale a_r = sqrt(max|rhs_r| / max|lhs_r|) (scales cancel in the
    product), packed as one DoubleRow matmul (2x128 rows: k2k3 | emb 0:128,
    double contraction per pass) plus a 72-row tail matmul
  - output: sigmoid encoded as uint8 via round(127.49*tanh(x/2) + 127.5)
    (decoded on host), quartering the store traffic vs f32

The entity axis is padded 1875 -> 1888 per core so every SBUF byte offset
the PE reads (slice starts, DoubleRow group stride) is 32B-aligned in
every dtype -- the tensor engine silently reads garbage at unaligned
per-partition offsets.

Device schedule: dummy warmup matmuls keep the PE busy (ramping its DVFS
clock) while inputs stream in as column halves, need-ordered and balanced
over the sync and gpsimd DMA queues; real matmuls then run slice-by-slice
with each PSUM bank's tanh (scalar) and u8 pack (vector) overlapping the
remaining matmuls, and stores batched per slice-pair on the sync queue.

Sharding: entities (E=15000) split evenly across 8 cores (1875 each);
batch side replicated; outputs concatenated on host.
"""

import sys

import numpy as np
import ml_dtypes

for _p in ("/opt/trn_rl_repo", "/root/.axon_site/_ro/trn_rl_repo"):
    if _p not in sys.path:
        sys.path.append(_p)

import concourse.bass as bass
import concourse.bacc as bacc
import concourse.mybir as mybir
from concourse import tile
from concourse import bass_utils

B, E, R, D, L = 256, 15000, 237, 200, 64
NCORES = 8
ES = E // NCORES          # 1875 entities per core
ESP = 1888                # padded per-core entity count (32B aligned all dtypes)
DEG = 3                   # polynomial degree for exp(x) on [-r_l, r_l]
KT = DEG + 1              # 4 polynomial terms
F32 = mybir.dt.float32
BF16 = mybir.dt.bfloat16
F8 = mybir.dt.float8e4
U8 = mybir.dt.uint8
BF16_NP = ml_dtypes.bfloat16
F8_NP = ml_dtypes.float8_e4m3fn
NF8 = 328                 # fp8 contraction rows: k2/k3 (128) + emb (200)
N_SLICES = [(0, 512), (512, 512), (1024, 512), (1504, 384)]
HALVES = [(0, 1024), (1024, ESP - 1024)]
N_WARM = 36               # dummy matmuls to ramp the PE clock
OSC, OBI = 127.49, 127.5  # u8 encode: round(OSC*tanh(x/2) + OBI)

TRACE = False             # test.py sets True to collect an NTFF profile
LAST = None               # last BassKernelResults (for test.py)

_PROG = None              # cached Bass program
_CHEB = None              # cached per-l polynomial coefficients


def _build_program():
    nc = bacc.Bacc("TRN2", target_bir_lowering=False, debug=False)

    rhs0_d = nc.dram_tensor("rhs0", [128, ESP], BF16, kind="ExternalInput")
    rhs12_d = nc.dram_tensor("rhs12", [128, 2, ESP], F8, kind="ExternalInput")
    rhs3_d = nc.dram_tensor("rhs3", [128, ESP], F8, kind="ExternalInput")
    lhs0_d = nc.dram_tensor("lhs0", [128, B], BF16, kind="ExternalInput")
    lhsf8_d = nc.dram_tensor("lhsf8", [128, 3, B], F8, kind="ExternalInput")
    out_d = nc.dram_tensor("out", [B, ESP], U8, kind="ExternalOutput")

    AF = mybir.ActivationFunctionType
    OP = mybir.AluOpType
    DR = mybir.MatmulPerfMode.DoubleRow

    with tile.TileContext(nc) as tc:
        with (
            tc.tile_pool(name="persist", bufs=1) as pool,
            tc.tile_pool(name="psum", bufs=1, space="PSUM") as ppool,
            tc.tile_pool(name="outs", bufs=4) as opool,
        ):
            # -- PE warmup: ramp the tensor-engine clock while DMAs run
            warm = pool.tile([128, 128], BF16, name="warm")
            wps = ppool.tile([128, 128], F32, name="wps", tag="wps", bufs=1)
            nc.vector.memset(warm, 0.0)
            for i in range(N_WARM):
                nc.tensor.matmul(wps, warm, warm, start=True, stop=True)

            # -- input DMAs, in need order, balanced across two queues
            lhs0 = pool.tile([128, B], BF16, name="lhs0")
            lhsf8 = pool.tile([128, 3, B], F8, name="lhsf8")
            rhs0 = pool.tile([128, ESP], BF16, name="rhs0")
            rhs12 = pool.tile([128, 2, ESP], F8, name="rhs12")
            rhs3 = pool.tile([128, ESP], F8, name="rhs3")
            (h0, hs0), (h1, hs1) = HALVES
            nc.sync.dma_start(lhs0, lhs0_d[:, :])
            nc.gpsimd.dma_start(lhsf8, lhsf8_d[:, :, :])
            nc.sync.dma_start(rhs0[:, h0 : h0 + hs0], rhs0_d[:, h0 : h0 + hs0])
            nc.gpsimd.dma_start(
                rhs12[:, :, h0 : h0 + hs0], rhs12_d[:, :, h0 : h0 + hs0]
            )
            nc.sync.dma_start(rhs3, rhs3_d[:, :])
            nc.gpsimd.dma_start(rhs0[:, h1 : h1 + hs1], rhs0_d[:, h1 : h1 + hs1])
            nc.sync.dma_start(
                rhs12[:, :, h1 : h1 + hs1], rhs12_d[:, :, h1 : h1 + hs1]
            )

            # -- matmul / sigmoid-as-u8, slice by slice; store per slice-pair
            obs = {}
            for si, (n0, nsz) in enumerate(N_SLICES):
                for m in range(2):
                    ms = np.s_[m * 128 : (m + 1) * 128]
                    ns = np.s_[n0 : n0 + nsz]
                    ps = ppool.tile([128, 512], F32, name="ps", tag="ps", bufs=4)
                    nc.tensor.matmul(
                        ps[:, :nsz], lhs0[:, ms], rhs0[:, ns],
                        start=True, stop=False,
                    )
                    nc.tensor.matmul(
                        ps[:, :nsz], lhsf8[:, 0:2, ms], rhs12[:, :, ns],
                        start=False, stop=False, perf_mode=DR,
                    )
                    nc.tensor.matmul(
                        ps[:, :nsz], lhsf8[:72, 2, ms], rhs3[:72, ns],
                        start=False, stop=True,
                    )
                    # u8 rows accumulate in a [128, 1024] buffer per (pair, m)
                    key = (si // 2, m)
                    if key not in obs:
                        obs[key] = opool.tile(
                            [128, 1024], U8, name=f"ob{key[0]}{m}", tag=f"ob{key[0]}{m}", bufs=1
                        )
                    ob = obs[key]
                    off = n0 - (si // 2) * 1024
                    tob = opool.tile([128, 512], BF16, name="tob", tag="tob")
                    nc.scalar.activation(tob[:, :nsz], ps[:, :nsz], AF.Tanh, scale=0.5)
                    nc.vector.tensor_scalar(
                        ob[:, off : off + nsz], tob[:, :nsz], OSC, OBI,
                        OP.mult, OP.add,
                    )
                    if si % 2 == 1:
                        p0 = (si // 2) * 1024
                        psz = n0 + nsz - p0
                        nc.sync.dma_start(
                            out_d[ms, p0 : p0 + psz], ob[:, :psz]
                        )

    nc.compile()
    return nc


def _host_prep(emb_e, emb_rel, nf_weights, lit, c, var, e1, rel):
    global _CHEB
    f64 = np.float64
    e1 = np.asarray(e1).astype(np.int64)
    rel = np.asarray(rel).astype(np.int64)
    lit64 = np.asarray(lit, f64)
    c64 = np.asarray(c, f64)
    var64 = np.asarray(var, f64)

    s = np.sqrt(var64)
    g = -c64 / s
    if _CHEB is None:
        r = 0.5 / var64
        C = np.zeros((KT, L))
        for l in range(L):
            ch = np.polynomial.chebyshev.Chebyshev.interpolate(
                np.exp, DEG, domain=[-r[l], r[l]]
            )
            C[:, l] = ch.convert(kind=np.polynomial.Polynomial).coef
        _CHEB = C
    C = _CHEB

    P = lit64[e1]                                   # (B, L)
    w = np.asarray(nf_weights, f64)[rel]            # (B, L)
    amg = (P - 0.5) / s                             # alpha - g
    alpha = amg + g
    u = w * np.exp(-(alpha**2))                     # (B, L)

    # polynomial-term factors
    A = np.empty((KT * L, B), f64)                  # (k-major rows, B)
    Bt = np.empty((KT * L, E), f64)
    beta = (lit64 - 0.5) / s                        # (E, L)
    V = np.exp(-((beta - g) ** 2) + g**2)           # (E, L)
    t2k = np.ones_like(amg)
    bk = V.copy()
    for k in range(KT):
        A[k * L : (k + 1) * L] = (u * C[k] * t2k).T
        Bt[k * L : (k + 1) * L] = bk.T
        t2k *= 2.0 * amg
        bk *= beta

    X = np.asarray(emb_e, f64)[e1] * np.asarray(emb_rel, f64)[rel]  # (B, D)

    # fp8 rows (k2,k3 + emb) with joint per-row rescale (cancels in product)
    Lr = np.concatenate([A[128:256], X.T], axis=0)        # (NF8, B)
    Rr = np.concatenate([Bt[128:256], np.asarray(emb_e, f64).T], axis=0)  # (NF8, E)
    mL = np.abs(Lr).max(axis=1)
    mR = np.abs(Rr).max(axis=1)
    mL[mL == 0] = 1.0
    mR[mR == 0] = 1.0
    a = np.sqrt(mR / mL)
    Lq = (Lr * a[:, None]).astype(F8_NP)                  # (NF8, B)
    Rq = (Rr / a[:, None]).astype(F8_NP)                  # (NF8, E)

    lhs0 = np.ascontiguousarray(A[:128].astype(BF16_NP))  # (128, B) bf16
    lhsf8 = np.zeros((128, 3, B), F8_NP)
    lhsf8[:, 0, :] = Lq[0:128]
    lhsf8[:, 1, :] = Lq[128:256]
    lhsf8[:72, 2, :] = Lq[256:NF8]

    rhs0 = np.zeros((128, NCORES, ESP), BF16_NP)
    rhs12 = np.zeros((128, 2, NCORES, ESP), F8_NP)
    rhs3 = np.zeros((128, NCORES, ESP), F8_NP)
    rhs0[:, :, :ES] = Bt[:128].astype(BF16_NP).reshape(128, NCORES, ES)
    rhs12[:, 0, :, :ES] = Rq[0:128].reshape(128, NCORES, ES)
    rhs12[:, 1, :, :ES] = Rq[128:256].reshape(128, NCORES, ES)
    rhs3[:72, :, :ES] = Rq[256:NF8].reshape(72, NCORES, ES)

    in_maps = []
    for ci in range(NCORES):
        in_maps.append(
            {
                "rhs0": np.ascontiguousarray(rhs0[:, ci]),
                "rhs12": np.ascontiguousarray(rhs12[:, :, ci]),
                "rhs3": np.ascontiguousarray(rhs3[:, ci]),
                "lhs0": lhs0,
                "lhsf8": lhsf8,
            }
        )
    return in_maps


def kernel(emb_e, emb_rel, nf_weights, lit, c, var, e1, rel):
    global _PROG, LAST
    if _PROG is None:
        _PROG = _build_program()
    in_maps = _host_prep(emb_e, emb_rel, nf_weights, lit, c, var, e1, rel)
    res = bass_utils.run_bass_kernel_spmd(
        _PROG, in_maps, core_ids=list(range(NCORES)), trace=TRACE
    )
    LAST = res
    q = np.concatenate(
        [res.results[ci]["out"][:, :ES] for ci in range(NCORES)], axis=1
    ).astype(np.float32)
    # decode u8: stored = round(OSC*tanh(x/2) + OBI) -> sigmoid = (tanh+1)/2
    t = (q - OBI) / OSC
    return np.clip((t + 1.0) * 0.5, 0.0, 1.0).astype(np.float32)


# revision 20
# speedup vs baseline: 1.2237x; 1.0896x over previous
"""Trainium2 Bass kernel for the KBLN scoring model.

Computes, for full inputs:
    score_l = (emb_e[e1] * emb_rel[rel]) @ emb_e.T                       (B, E)
    phi     = exp(-((lit[e1][:,None,:] - lit[None,:,:]) - c)^2 / var)    (B, E, L)
    score_n = einsum('bel,bl->be', phi, nf_weights[rel])
    out     = sigmoid(score_l + score_n)

Reformulation
-------------
With alpha[b,l] = (lit[e1[b],l] - 0.5 - c[l]) / sqrt(var[l]),
     beta[e,l]  = (lit[e,l]    - 0.5)        / sqrt(var[l]),
     g[l]       = -c[l] / sqrt(var[l]):

    phi = exp(-alpha^2) * exp(-(beta-g)^2 + g^2) * exp(x),
    x   = 2*(alpha-g)*beta,  |x| <= 0.5/var[l]  (per-l bound r_l <= 1).

exp(x) is replaced by a per-l degree-3 Chebyshev-interpolant polynomial on
[-r_l, r_l] (near-minimax), which makes score_n a matmul with contraction
4*64 = 256:

    score_n[b,e] = sum_{k,l} A[b,(k,l)] * Bt[(k,l),e]
    A[b,(k,l)]  = w * exp(-alpha^2) * C[k,l] * (2*(alpha-g))^k    (host, tiny)
    Bt[(k,l),e] = exp(-(beta-g)^2 + g^2) * beta^k                 (host, E*L)

score_l folds in as 200 extra contraction rows.  All feature construction
happens on host (it is O(E*L), tiny next to the O(B*E*L) matmul), so the
device program is pure DMA + matmul + sigmoid.

Precision split (tolerance is 2e-2; this lands ~7e-3):
  - k0/k1 polynomial rows (the dominant terms): bf16 x bf16 matmul
  - k2/k3 rows + all 200 emb rows: fp8(e4m3) on BOTH sides with a per-row
    joint rescale a_r = sqrt(max|rhs_r| / max|lhs_r|) (scales cancel in the
    product), packed as one DoubleRow matmul (2x128 rows: k2k3 | emb 0:128,
    double contraction per pass) plus a 72-row tail matmul
  - output: sigmoid encoded as uint8 via round(127.49*tanh(x/2) + 127.5)
    (decoded on host), quartering the store traffic vs f32

The entity axis is padded 1875 -> 1888 per core so every SBUF byte offset
the PE reads (slice starts, DoubleRow group stride) is 32B-aligned in
every dtype -- the tensor engine silently reads garbage at unaligned
per-partition offsets.

Device schedule: dummy warmup matmuls keep the PE busy (ramping its DVFS
clock) while inputs stream in as column halves, need-ordered and balanced
over the sync and gpsimd DMA queues; real matmuls then run slice-by-slice
with each PSUM bank's tanh (scalar) and u8 pack (vector) overlapping the
remaining matmuls, and stores batched per slice-pair on the sync queue.

Sharding: entities (E=15000) split evenly across 8 cores (1875 each);
batch side replicated; outputs concatenated on host.
"""

import sys

import numpy as np
import ml_dtypes

for _p in ("/opt/trn_rl_repo", "/root/.axon_site/_ro/trn_rl_repo"):
    if _p not in sys.path:
        sys.path.append(_p)

import concourse.bass as bass
import concourse.bacc as bacc
import concourse.mybir as mybir
from concourse import tile
from concourse import bass_utils

B, E, R, D, L = 256, 15000, 237, 200, 64
NCORES = 8
ES = E // NCORES          # 1875 entities per core
ESP = 1888                # padded per-core entity count (32B aligned all dtypes)
DEG = 3                   # polynomial degree for exp(x) on [-r_l, r_l]
KT = DEG + 1              # 4 polynomial terms
F32 = mybir.dt.float32
BF16 = mybir.dt.bfloat16
F8 = mybir.dt.float8e4
U8 = mybir.dt.uint8
BF16_NP = ml_dtypes.bfloat16
F8_NP = ml_dtypes.float8_e4m3fn
NF8 = 328                 # fp8 contraction rows: k2/k3 (128) + emb (200)
N_SLICES = [(0, 512), (512, 512), (1024, 512), (1504, 384)]
HALVES = [(0, 1024), (1024, ESP - 1024)]
N_WARM = 46               # dummy matmuls to ramp the PE clock
OSC, OBI = 127.49, 127.5  # u8 encode: round(OSC*tanh(x/2) + OBI)

TRACE = False             # test.py sets True to collect an NTFF profile
LAST = None               # last BassKernelResults (for test.py)

_PROG = None              # cached Bass program
_CHEB = None              # cached per-l polynomial coefficients


def _build_program():
    nc = bacc.Bacc("TRN2", target_bir_lowering=False, debug=False)

    rhs0_d = nc.dram_tensor("rhs0", [128, ESP], BF16, kind="ExternalInput")
    rhs12_d = nc.dram_tensor("rhs12", [128, 2, ESP], F8, kind="ExternalInput")
    rhs3_d = nc.dram_tensor("rhs3", [128, ESP], F8, kind="ExternalInput")
    lhs0_d = nc.dram_tensor("lhs0", [128, B], BF16, kind="ExternalInput")
    lhsf8_d = nc.dram_tensor("lhsf8", [128, 3, B], F8, kind="ExternalInput")
    out_d = nc.dram_tensor("out", [B, ESP], U8, kind="ExternalOutput")

    AF = mybir.ActivationFunctionType
    OP = mybir.AluOpType
    DR = mybir.MatmulPerfMode.DoubleRow

    with tile.TileContext(nc) as tc:
        with (
            tc.tile_pool(name="persist", bufs=1) as pool,
            tc.tile_pool(name="psum", bufs=1, space="PSUM") as ppool,
            tc.tile_pool(name="outs", bufs=4) as opool,
        ):
            # -- PE warmup: ramp the tensor-engine clock while DMAs run
            warm = pool.tile([128, 128], BF16, name="warm")
            wps = ppool.tile([128, 128], F32, name="wps", tag="wps", bufs=1)
            nc.vector.memset(warm, 0.0)
            for i in range(N_WARM):
                nc.tensor.matmul(wps, warm, warm, start=True, stop=True)

            # -- input DMAs, in need order, balanced across two queues
            lhs0 = pool.tile([128, B], BF16, name="lhs0")
            lhsf8 = pool.tile([128, 3, B], F8, name="lhsf8")
            rhs0 = pool.tile([128, ESP], BF16, name="rhs0")
            rhs12 = pool.tile([128, 2, ESP], F8, name="rhs12")
            rhs3 = pool.tile([128, ESP], F8, name="rhs3")
            (h0, hs0), (h1, hs1) = HALVES
            nc.sync.dma_start(rhs0[:, h0 : h0 + hs0], rhs0_d[:, h0 : h0 + hs0])
            nc.gpsimd.dma_start(lhs0, lhs0_d[:, :])
            nc.gpsimd.dma_start(lhsf8, lhsf8_d[:, :, :])
            nc.gpsimd.dma_start(
                rhs12[:, :, h0 : h0 + hs0], rhs12_d[:, :, h0 : h0 + hs0]
            )
            nc.sync.dma_start(rhs3, rhs3_d[:, :])
            nc.gpsimd.dma_start(rhs0[:, h1 : h1 + hs1], rhs0_d[:, h1 : h1 + hs1])
            nc.sync.dma_start(
                rhs12[:, :, h1 : h1 + hs1], rhs12_d[:, :, h1 : h1 + hs1]
            )

            # -- matmul / sigmoid-as-u8, slice by slice; store per slice-pair
            obs = {}
            for si, (n0, nsz) in enumerate(N_SLICES):
                for m in range(2):
                    ms = np.s_[m * 128 : (m + 1) * 128]
                    ns = np.s_[n0 : n0 + nsz]
                    ps = ppool.tile([128, 512], F32, name="ps", tag="ps", bufs=4)
                    nc.tensor.matmul(
                        ps[:, :nsz], lhs0[:, ms], rhs0[:, ns],
                        start=True, stop=False,
                    )
                    nc.tensor.matmul(
                        ps[:, :nsz], lhsf8[:, 0:2, ms], rhs12[:, :, ns],
                        start=False, stop=False, perf_mode=DR,
                    )
                    nc.tensor.matmul(
                        ps[:, :nsz], lhsf8[:72, 2, ms], rhs3[:72, ns],
                        start=False, stop=True,
                    )
                    # u8 rows accumulate in a [128, 1024] buffer per (pair, m)
                    key = (si // 2, m)
                    if key not in obs:
                        obs[key] = opool.tile(
                            [128, 1024], U8, name=f"ob{key[0]}{m}", tag=f"ob{key[0]}{m}", bufs=1
                        )
                    ob = obs[key]
                    off = n0 - (si // 2) * 1024
                    tob = opool.tile([128, 512], BF16, name="tob", tag="tob")
                    nc.scalar.activation(tob[:, :nsz], ps[:, :nsz], AF.Tanh, scale=0.5)
                    nc.vector.tensor_scalar(
                        ob[:, off : off + nsz], tob[:, :nsz], OSC, OBI,
                        OP.mult, OP.add,
                    )
                    if si % 2 == 1:
                        p0 = (si // 2) * 1024
                        psz = n0 + nsz - p0
                        nc.sync.dma_start(
                            out_d[ms, p0 : p0 + psz], ob[:, :psz]
                        )

    nc.compile()
    return nc


def _host_prep(emb_e, emb_rel, nf_weights, lit, c, var, e1, rel):
    global _CHEB
    f64 = np.float64
    e1 = np.asarray(e1).astype(np.int64)
    rel = np.asarray(rel).astype(np.int64)
    lit64 = np.asarray(lit, f64)
    c64 = np.asarray(c, f64)
    var64 = np.asarray(var, f64)

    s = np.sqrt(var64)
    g = -c64 / s
    if _CHEB is None:
        r = 0.5 / var64
        C = np.zeros((KT, L))
        for l in range(L):
            ch = np.polynomial.chebyshev.Chebyshev.interpolate(
                np.exp, DEG, domain=[-r[l], r[l]]
            )
            C[:, l] = ch.convert(kind=np.polynomial.Polynomial).coef
        _CHEB = C
    C = _CHEB

    P = lit64[e1]                                   # (B, L)
    w = np.asarray(nf_weights, f64)[rel]            # (B, L)
    amg = (P - 0.5) / s                             # alpha - g
    alpha = amg + g
    u = w * np.exp(-(alpha**2))                     # (B, L)

    # polynomial-term factors
    A = np.empty((KT * L, B), f64)                  # (k-major rows, B)
    Bt = np.empty((KT * L, E), f64)
    beta = (lit64 - 0.5) / s                        # (E, L)
    V = np.exp(-((beta - g) ** 2) + g**2)           # (E, L)
    t2k = np.ones_like(amg)
    bk = V.copy()
    for k in range(KT):
        A[k * L : (k + 1) * L] = (u * C[k] * t2k).T
        Bt[k * L : (k + 1) * L] = bk.T
        t2k *= 2.0 * amg
        bk *= beta

    X = np.asarray(emb_e, f64)[e1] * np.asarray(emb_rel, f64)[rel]  # (B, D)

    # fp8 rows (k2,k3 + emb) with joint per-row rescale (cancels in product)
    Lr = np.concatenate([A[128:256], X.T], axis=0)        # (NF8, B)
    Rr = np.concatenate([Bt[128:256], np.asarray(emb_e, f64).T], axis=0)  # (NF8, E)
    mL = np.abs(Lr).max(axis=1)
    mR = np.abs(Rr).max(axis=1)
    mL[mL == 0] = 1.0
    mR[mR == 0] = 1.0
    a = np.sqrt(mR / mL)
    Lq = (Lr * a[:, None]).astype(F8_NP)                  # (NF8, B)
    Rq = (Rr / a[:, None]).astype(F8_NP)                  # (NF8, E)

    lhs0 = np.ascontiguousarray(A[:128].astype(BF16_NP))  # (128, B) bf16
    lhsf8 = np.zeros((128, 3, B), F8_NP)
    lhsf8[:, 0, :] = Lq[0:128]
    lhsf8[:, 1, :] = Lq[128:256]
    lhsf8[:72, 2, :] = Lq[256:NF8]

    rhs0 = np.zeros((128, NCORES, ESP), BF16_NP)
    rhs12 = np.zeros((128, 2, NCORES, ESP), F8_NP)
    rhs3 = np.zeros((128, NCORES, ESP), F8_NP)
    rhs0[:, :, :ES] = Bt[:128].astype(BF16_NP).reshape(128, NCORES, ES)
    rhs12[:, 0, :, :ES] = Rq[0:128].reshape(128, NCORES, ES)
    rhs12[:, 1, :, :ES] = Rq[128:256].reshape(128, NCORES, ES)
    rhs3[:72, :, :ES] = Rq[256:NF8].reshape(72, NCORES, ES)

    in_maps = []
    for ci in range(NCORES):
        in_maps.append(
            {
                "rhs0": np.ascontiguousarray(rhs0[:, ci]),
                "rhs12": np.ascontiguousarray(rhs12[:, :, ci]),
                "rhs3": np.ascontiguousarray(rhs3[:, ci]),
                "lhs0": lhs0,
                "lhsf8": lhsf8,
            }
        )
    return in_maps


def kernel(emb_e, emb_rel, nf_weights, lit, c, var, e1, rel):
    global _PROG, LAST
    if _PROG is None:
        _PROG = _build_program()
    in_maps = _host_prep(emb_e, emb_rel, nf_weights, lit, c, var, e1, rel)
    res = bass_utils.run_bass_kernel_spmd(
        _PROG, in_maps, core_ids=list(range(NCORES)), trace=TRACE
    )
    LAST = res
    q = np.concatenate(
        [res.results[ci]["out"][:, :ES] for ci in range(NCORES)], axis=1
    ).astype(np.float32)
    # decode u8: stored = round(OSC*tanh(x/2) + OBI) -> sigmoid = (tanh+1)/2
    t = (q - OBI) / OSC
    return np.clip((t + 1.0) * 0.5, 0.0, 1.0).astype(np.float32)
